# revision 1
# baseline (speedup 1.0000x reference)
"""ChebConv (K=4) GNN message passing on 8 Trainium2 NeuronCores.

Strategy (1D vertex partitioning, hardcoded for N=100000, E=1600000,
D_in=D_out=64, K=4, lambda_max=2.0):

  Node re-sharding: nodes are padded to NPAD=102400 and mapped
  n -> (g = n // 25600, j = n % 25600, c = j // 3200, r = j % 3200).
  Core c owns local row l = g*3200 + r (SHARD=12800 rows, 100 blocks of
  128). Groups g are the gather slabs: slab_g = AllGather over cores of
  their [3200, 64] group-g rows, giving slab row j = c*3200 + r < 25600
  (int16-safe gather indices).

  Per propagation (x' = Tx * norm):
    - xn_g is computed per group with one batched vector op, DMA'd to
      DRAM and AllGathered per group. The xn+AllGather for prop p+1 is
      pre-issued inside prop p's span loop as soon as the spans covering
      group g's blocks finish their recurrence, hiding the collectives.
    - Edges are partitioned by dst owner and sorted by (src_group,
      dst_block); each 128-edge tile is fetched with chunked dma_gather
      (2048 idxs/call, round-robin over the 4 SWDGE queues; issue order
      is front-load-then-round-robin across groups so the in-order Pool
      engine never gates one group's prefetch on another group's
      consumption).
    - Selection matrices (sel[e,d] = local_dst[e]==d) are generated 6
      tiles per vector instruction via dual multi-dim broadcast
      is_equal, just-in-time in consumption order (per-group rings).
    - Segment-sum: per 8-dst-block span, one PSUM bank accumulates each
      block's tiles from all 4 groups in a single uninterrupted matmul
      chain (interleaved open chains on a bank corrupt accumulation on
      real HW). The drain is fused into the Chebyshev recurrence: the
      vector engine reads the bank directly (Tx1 = -h*norm;
      Tx_i = -2h*norm - prev), batched per span.
  Final: rst^T = sum_k W_k^T @ Tx_k^T per block via PE transposes batched
  4 blocks per PSUM bank, f32 W matmuls, fused bias via tensor_scalar.

The host permutes feat/deg in and un-permutes rstT out.
"""
import numpy as np
from contextlib import ExitStack

import concourse.bass as bass
import concourse.tile as tile
from concourse import bacc, mybir
from concourse.bass_utils import run_bass_kernel_spmd
from concourse.masks import make_identity

N = 100000
E = 1600000
D = 64
KORD = 4
NCORES = 8
P = 128
NGRP = 4
PER = 3200              # rows per core per group
GPB = PER // P          # 25 blocks per group
NBLK = NGRP * GPB       # 100 blocks per core
SHARD = NGRP * PER      # 12800 rows per core
GRP = NCORES * PER      # 25600 rows per slab (int16-safe)
NPAD = NGRP * GRP       # 102400
CHUNK_TILES = 16        # 2048 idxs per dma_gather call
MSG_BUFS = 3            # msg ring depth per source group
SELT = 6                # max tiles per batched sel-gen instruction
SPAN = 8                # dst blocks per PSUM bank span
PAD_SELCOL = 999.0


def _node_map(n):
    """global node id -> (core, local row)"""
    g = n // GRP
    j = n % GRP
    c = j // PER
    r = j % PER
    return c, g * PER + r


def _prepare_edges(src: np.ndarray, dst: np.ndarray):
    """Partition + sort edges per core; build idx16/selcol streams."""
    dg = dst // GRP
    dj = dst % GRP
    owner = dj // PER
    ldst = dg * PER + dj % PER          # local row on owner core
    gsrc = src // GRP
    lsrc = src % GRP                    # slab-local row
    b = ldst >> 7

    per_core = []
    counts = np.zeros((NCORES, NGRP, NBLK), dtype=np.int64)
    for c in range(NCORES):
        m = owner == c
        s, d, g, bb = lsrc[m], ldst[m], gsrc[m], b[m]
        order = np.lexsort((d, bb, g))
        s, d, g, bb = s[order], d[order], g[order], bb[order]
        np.add.at(counts[c], (g, bb), 1)
        per_core.append((s, d, g, bb))
    tiles_per = np.maximum(1, (counts.max(axis=0) + 127) // 128)  # [NGRP, NBLK]
    ntile = int(tiles_per.sum())
    nslot = ntile * P
    tile_start = np.zeros((NGRP, NBLK), dtype=np.int64)
    acc = 0
    for g in range(NGRP):
        for bb in range(NBLK):
            tile_start[g, bb] = acc
            acc += tiles_per[g, bb]

    idx_streams, sel_streams = [], []
    for c in range(NCORES):
        s, d, g, bb = per_core[c]
        flat_idx = np.zeros(nslot, dtype=np.int16)
        flat_sel = np.full(nslot, PAD_SELCOL, dtype=np.float32)
        cnt = counts[c]
        estart = np.concatenate([[0], np.cumsum(cnt.ravel())])
        for gg in range(NGRP):
            for bb2 in range(NBLK):
                k = gg * NBLK + bb2
                e0, e1 = estart[k], estart[k + 1]
                if e1 == e0:
                    continue
                s0 = tile_start[gg, bb2] * P
                n = e1 - e0
                flat_idx[s0:s0 + n] = s[e0:e1].astype(np.int16)
                flat_sel[s0:s0 + n] = (d[e0:e1] & 127).astype(np.float32)
        idx16 = np.tile(flat_idx.reshape(nslot // 16, 16).T, (8, 1))
        selcol = np.ascontiguousarray(flat_sel.reshape(ntile, P).T)
        idx_streams.append(np.ascontiguousarray(idx16))
        sel_streams.append(selcol)
    return tiles_per, tile_start, ntile, idx_streams, sel_streams


def _build_nc(tiles_per, tile_start, ntile, ablate=()):
    """ablate: dev-only subsets of {'nogather','noselmm','nocoll','q1',
    'nodrain'} for component timing; production call passes nothing."""
    ablate = frozenset(ablate)
    nc = bacc.Bacc("TRN2", target_bir_lowering=False, debug=False,
                   enable_asserts=True, num_devices=NCORES,
                   num_swdge_queues=4)
    f32 = mybir.dt.float32
    bf16 = mybir.dt.bfloat16
    nslot16 = ntile * P // 16

    feat_in = nc.dram_tensor("feat", [SHARD, D], f32, kind="ExternalInput").ap()
    deg_in = nc.dram_tensor("deg", [P, NBLK], f32, kind="ExternalInput").ap()
    idx_in = nc.dram_tensor("idx", [P, nslot16], mybir.dt.int16, kind="ExternalInput").ap()
    sel_in = nc.dram_tensor("selcol", [P, ntile], bf16, kind="ExternalInput").ap()
    iota_in = nc.dram_tensor("iota", [P, P], bf16, kind="ExternalInput").ap()
    w_in = nc.dram_tensor("w", [D, KORD * D], f32, kind="ExternalInput").ap()
    b_in = nc.dram_tensor("bias", [D, 1], f32, kind="ExternalInput").ap()
    out = nc.dram_tensor("rstT", [D, SHARD], f32, kind="ExternalOutput").ap()

    # span layout over the 100 blocks: 12 spans of 8 + 1 span of 4
    spans = []
    b0 = 0
    while b0 < NBLK:
        spans.append((b0, min(b0 + SPAN, NBLK)))
        b0 += SPAN
    # last span fully covering each group's final block
    grp_last_span = [max(si for si, (s0, s1) in enumerate(spans)
                         if s0 < (g + 1) * GPB) for g in range(NGRP)]

    with tile.TileContext(nc) as tc:
        with ExitStack() as ctx:
            const = ctx.enter_context(tc.tile_pool(name="const", bufs=1))
            txp = ctx.enter_context(tc.tile_pool(name="txp", bufs=1))
            xnp = ctx.enter_context(tc.tile_pool(name="xnp", bufs=2))
            msgp = ctx.enter_context(tc.tile_pool(name="msgp", bufs=MSG_BUFS))
            selp = ctx.enter_context(tc.tile_pool(name="selp", bufs=2))
            psum = ctx.enter_context(tc.tile_pool(name="psum", bufs=4, space="PSUM"))
            trp = ctx.enter_context(tc.tile_pool(name="trp", bufs=2, space="PSUM"))
            rstp = ctx.enter_context(tc.tile_pool(name="rstp", bufs=2, space="PSUM"))
            outp = ctx.enter_context(tc.tile_pool(name="outp", bufs=1))
            featp = ctx.enter_context(tc.tile_pool(name="featp", bufs=2))
            dram = ctx.enter_context(tc.tile_pool(name="dram", bufs=1, space="DRAM"))

            # ---- constants / streams
            idx_sb = const.tile([P, nslot16], mybir.dt.int16)
            nc.sync.dma_start(idx_sb[:], idx_in[:])
            sel_sb = const.tile([P, ntile], bf16)
            nc.sync.dma_start(sel_sb[:], sel_in[:])
            iota_sb = const.tile([P, P], bf16)
            nc.sync.dma_start(iota_sb[:], iota_in[:])
            w_sb = const.tile([D, KORD * D], f32)
            nc.sync.dma_start(w_sb[:], w_in[:])
            b_sb = const.tile([D, 1], f32)
            nc.sync.dma_start(b_sb[:], b_in[:])
            ident = const.tile([P, P], f32)
            make_identity(nc, ident[:])

            # ---- norm = 1/sqrt(max(deg,1)); nnorm = -norm; n2norm = -2*norm
            deg_sb = const.tile([P, NBLK], f32)
            nc.sync.dma_start(deg_sb[:], deg_in[:])
            norm = const.tile([P, NBLK], f32)
            nc.vector.tensor_scalar_max(norm[:], deg_sb[:], 1.0)
            nc.scalar.activation(norm[:], norm[:], mybir.ActivationFunctionType.Sqrt)
            nc.vector.reciprocal(norm[:], norm[:])
            nnorm = const.tile([P, NBLK], f32)
            nc.vector.tensor_scalar_mul(nnorm[:], norm[:], -1.0)
            n2norm = const.tile([P, NBLK], f32)
            nc.vector.tensor_scalar_mul(n2norm[:], norm[:], -2.0)

            # ---- Tx ring buffers (node-major [p, b*64+f])
            txA = txp.tile([P, NBLK * D], f32)   # feat -> later Tx3
            txB = txp.tile([P, NBLK * D], f32)   # Tx1
            txC = txp.tile([P, NBLK * D], f32)   # Tx2
            nc.sync.dma_start(
                txA[:].rearrange("p (b f) -> p b f", b=NBLK),
                feat_in.rearrange("(b p) f -> p b f", p=P))

            table_own = [[dram.tile([PER, D], f32, name=f"town{pp}_{g}")
                          for g in range(NGRP)] for pp in range(KORD - 1)]
            slabs = [[dram.tile([GRP, D], f32, addr_space="Shared",
                                name=f"slab{pp}_{g}")
                      for g in range(NGRP)] for pp in range(KORD - 1)]
            tx_ring = [txA, txB, txC]

            def emit_xn_ag(pp, g, tx_src):
                """xn_g = tx_src[group g] * norm -> DRAM -> AllGather slab."""
                c0, c1 = g * GPB * D, (g + 1) * GPB * D
                xn_t = xnp.tile([P, GPB * D], f32, tag="xn",
                                name=f"xn{pp}_{g}")
                nc.vector.tensor_tensor(
                    out=xn_t[:].rearrange("p (b f) -> p b f", b=GPB),
                    in0=tx_src[:, c0:c1].rearrange("p (b f) -> p b f", b=GPB),
                    in1=norm[:, g * GPB:(g + 1) * GPB]
                        .rearrange("p (b o) -> p b o", o=1)
                        .to_broadcast([P, GPB, D]),
                    op=mybir.AluOpType.mult,
                )
                nc.sync.dma_start(
                    table_own[pp - 1][g].opt().rearrange("(b p) f -> p b f", p=P),
                    xn_t[:].rearrange("p (b f) -> p b f", b=GPB))
                if "nocoll" in ablate:
                    nc.sync.dma_start(
                        slabs[pp - 1][g].opt()[0:PER, :],
                        table_own[pp - 1][g].opt()[:, :])
                else:
                    nc.gpsimd.collective_compute(
                        "AllGather", mybir.AluOpType.bypass,
                        replica_groups=[list(range(NCORES))],
                        ins=[table_own[pp - 1][g].opt()],
                        outs=[slabs[pp - 1][g].opt()],
                    )

            for g in range(NGRP):
                emit_xn_ag(1, g, txA)

            for prop in range(1, KORD):
                tx_cur = tx_ring[(prop - 1) % 3]
                tx_new = tx_ring[prop % 3]
                tx_prev2 = tx_ring[(prop - 2) % 3]
                if "noselmm" in ablate or "nodrain" in ablate:
                    nc.vector.memset(tx_new[:], 0.0)
                if "agtop" in ablate and prop > 1:
                    for g in range(NGRP):
                        emit_xn_ag(prop, g, tx_cur)

                # gathers for all groups (waits AG_g via slab deps); issue
                # order is front-load-then-round-robin so the in-order Pool
                # engine never gates one group's prefetch on another group's
                # consumption (deadlocks otherwise)
                chunk_lists = []
                for g in range(NGRP):
                    gt0 = int(tile_start[g, 0])
                    gt1 = int(tile_start[g, NBLK - 1] + tiles_per[g, NBLK - 1])
                    cl = []
                    j = gt0
                    while j < gt1:
                        cnt = min(CHUNK_TILES, gt1 - j)
                        cl.append((j, cnt))
                        j += cnt
                    chunk_lists.append(cl)
                issue = []
                FRONT = 3
                for g in range(NGRP):
                    for c in range(min(FRONT, len(chunk_lists[g]))):
                        issue.append((g, c))
                rrmax = max(len(cl) for cl in chunk_lists)
                for c in range(FRONT, rrmax):
                    for g in range(NGRP):
                        if c < len(chunk_lists[g]):
                            issue.append((g, c))
                chunk_tiles = {}
                for qi, (g, ci) in enumerate(issue):
                    j, cnt = chunk_lists[g][ci]
                    tbl = slabs[prop - 1][g].opt()
                    m = msgp.tile([P, CHUNK_TILES, D], f32, tag=f"msg{g}",
                                  name=f"m{prop}_{g}_{ci}")
                    if "nogather" in ablate:
                        nc.vector.memset(m[:, 0:cnt, :], 0.25)
                    else:
                        nc.gpsimd.dma_gather(
                            out_ap=m[:, 0:cnt, :],
                            in_ap=tbl[:, :],
                            idxs_ap=idx_sb[:, j * 8:(j + cnt) * 8],
                            num_idxs=cnt * P,
                            num_idxs_reg=cnt * P,
                            elem_size=D,
                            single_packet="sp1" in ablate,
                            queue_num=0 if "q1" in ablate else qi % 4,
                        )
                    for jl in range(cnt):
                        chunk_tiles[j + jl] = (m, jl)

                if "noselmm" in ablate:
                    if prop < KORD - 1:
                        for g in range(NGRP):
                            emit_xn_ag(prop + 1, g, tx_new)
                    continue
                # span-major, block-contiguous chains: for each block the 4
                # groups' tiles accumulate in ONE uninterrupted PSUM chain
                # (interleaved open chains on a bank corrupt accumulation on
                # HW); sel batches are generated just-in-time in consumption
                # order; the drain is fused into the recurrence (DVE reads
                # the bank directly)
                for si, (s0, s1) in enumerate(spans):
                    bank = psum.tile([P, SPAN * D], f32, tag="ps",
                                     name=f"ps{prop}_{si}")
                    sel_tiles = {}
                    gen_pos = []
                    for g in range(NGRP):
                        gen_pos.append(int(tile_start[g, s0]))
                    span_end = [int(tile_start[g, s1 - 1] + tiles_per[g, s1 - 1])
                                for g in range(NGRP)]
                    for b in range(s0, s1):
                        for g in range(NGRP):
                            tb = int(tiles_per[g, b])
                            j0 = int(tile_start[g, b])
                            while gen_pos[g] < j0 + tb:
                                jg = gen_pos[g]
                                cnt = min(SELT, span_end[g] - jg)
                                sel = selp.tile([P, SELT, P], f32, tag=f"sel{g}",
                                                name=f"sel{prop}_{si}_{g}_{jg}")
                                nc.vector.tensor_tensor(
                                    out=sel[:, 0:cnt, :],
                                    in0=sel_sb[:, jg:jg + cnt]
                                        .rearrange("p (t o) -> p t o", o=1)
                                        .to_broadcast([P, cnt, P]),
                                    in1=iota_sb[:]
                                        .rearrange("p (o d) -> p o d", o=1)
                                        .to_broadcast([P, cnt, P]),
                                    op=mybir.AluOpType.is_equal,
                                )
                                for jl in range(cnt):
                                    sel_tiles[jg + jl] = (sel, jl)
                                gen_pos[g] = jg + cnt
                            sl = bank[:, (b - s0) * D:(b - s0 + 1) * D]
                            for t in range(tb):
                                jj = j0 + t
                                m, jl = chunk_tiles[jj]
                                sel, sjl = sel_tiles[jj]
                                nc.tensor.matmul(
                                    sl, lhsT=sel[:, sjl, :], rhs=m[:, jl, :],
                                    start=(g == 0 and t == 0),
                                    stop=(g == NGRP - 1 and t == tb - 1),
                                    skip_group_check=True,
                                )
                    if "nodrain" in ablate:
                        if prop < KORD - 1 and si == len(spans) - 1:
                            for g in range(NGRP):
                                emit_xn_ag(prop + 1, g, tx_new)
                        continue
                    nb = s1 - s0
                    scale = nnorm if prop == 1 else n2norm
                    nc.vector.tensor_tensor(
                        out=tx_new[:, s0 * D:s1 * D]
                            .rearrange("p (b f) -> p b f", b=nb),
                        in0=bank[:, 0:nb * D]
                            .rearrange("p (b f) -> p b f", b=nb),
                        in1=scale[:, s0:s1]
                            .rearrange("p (b o) -> p b o", o=1)
                            .to_broadcast([P, nb, D]),
                        op=mybir.AluOpType.mult,
                    )
                    if prop >= 2:
                        nc.vector.tensor_tensor(
                            out=tx_new[:, s0 * D:s1 * D],
                            in0=tx_new[:, s0 * D:s1 * D],
                            in1=tx_prev2[:, s0 * D:s1 * D],
                            op=mybir.AluOpType.subtract,
                        )
                    # pre-issue next prop's AllGather for any node-group whose
                    # blocks are now final, hiding the collective behind the
                    # remaining spans
                    if prop < KORD - 1 and "agtop" not in ablate:
                        for gq in range(NGRP):
                            if grp_last_span[gq] == si:
                                emit_xn_ag(prop + 1, gq, tx_new)

            # ---- rst^T = sum_k W_k^T @ Tx_k^T + b, 4 blocks per PSUM bank
            FB = 4  # blocks per output bank
            for q in range(NBLK // FB):
                blocks = range(q * FB, (q + 1) * FB)
                featb = featp.tile([P, FB, D], f32, tag="fb", name=f"fb{q}")
                nc.sync.dma_start(
                    featb[:],
                    feat_in[q * FB * P:(q + 1) * FB * P, :]
                        .rearrange("(b p) f -> p b f", p=P))
                rst_ps = rstp.tile([D, FB * P], f32, tag="rst", name=f"rst{q}")
                for bi, b in enumerate(blocks):
                    tr_ps = trp.tile([D, KORD * P], f32, tag="tr", name=f"tr{b}")
                    srcs = [featb[:, bi, :], txB[:, b * D:(b + 1) * D],
                            txC[:, b * D:(b + 1) * D], txA[:, b * D:(b + 1) * D]]
                    for k in range(KORD):
                        nc.tensor.transpose(
                            tr_ps[:, k * P:(k + 1) * P], srcs[k], ident[:])
                    txT = outp.tile([D, KORD * P], f32, tag="txT", name=f"txT{b}")
                    nc.scalar.activation(
                        txT[:], tr_ps[:], mybir.ActivationFunctionType.Copy)
                    for k in range(KORD):
                        nc.tensor.matmul(
                            rst_ps[:, bi * P:(bi + 1) * P],
                            lhsT=w_sb[:, k * D:(k + 1) * D],
                            rhs=txT[:, k * P:(k + 1) * P],
                            start=(k == 0), stop=(k == KORD - 1),
                            skip_group_check=True,
                        )
                ostage = outp.tile([D, FB * P], f32, tag="ostage", name=f"os{q}")
                nc.vector.tensor_scalar(
                    out=ostage[:], in0=rst_ps[:], scalar1=b_sb[:, 0:1],
                    scalar2=None, op0=mybir.AluOpType.add)
                nc.sync.dma_start(out[:, q * FB * P:(q + 1) * FB * P], ostage[:])
    nc.compile()
    return nc


_CACHE = {}


def _get_compiled(src: np.ndarray, dst: np.ndarray):
    key = (src.tobytes()[:256], dst.tobytes()[:256], len(src))
    if key not in _CACHE:
        tpb, ts, ntile, idx_s, sel_s = _prepare_edges(src, dst)
        nc = _build_nc(tpb, ts, ntile)
        _CACHE[key] = (nc, idx_s, sel_s)
    return _CACHE[key]


def _perm_rows():
    """perm[c, l] = global node id for core c local row l (or -1 if pad)."""
    l = np.arange(SHARD)
    g = l // PER
    r = l % PER
    perm = np.empty((NCORES, SHARD), dtype=np.int64)
    for c in range(NCORES):
        n = g * GRP + c * PER + r
        perm[c] = np.where(n < N, n, -1)
    return perm


_PERM = _perm_rows()


def _make_in_maps(feat, src, dst, W, b, idx_s, sel_s):
    deg_full = np.bincount(dst, minlength=N).astype(np.float32)
    iota = np.broadcast_to(np.arange(P, dtype=np.float32)[None, :], (P, P))
    w_flat = np.ascontiguousarray(
        W.astype(np.float32).transpose(1, 0, 2).reshape(D, KORD * D))
    b_col = np.ascontiguousarray(b.astype(np.float32).reshape(D, 1))
    try:
        import ml_dtypes
        bf = ml_dtypes.bfloat16
    except ImportError:
        bf = np.float32
    in_maps = []
    for c in range(NCORES):
        perm = _PERM[c]
        valid = perm >= 0
        feat_c = np.zeros((SHARD, D), dtype=np.float32)
        feat_c[valid] = feat[perm[valid]]
        deg_c = np.zeros(SHARD, dtype=np.float32)
        deg_c[valid] = deg_full[perm[valid]]
        in_maps.append({
            "feat": feat_c,
            "deg": np.ascontiguousarray(deg_c.reshape(NBLK, P).T),
            "idx": idx_s[c],
            "selcol": sel_s[c].astype(bf),
            "iota": iota.astype(bf),
            "w": w_flat,
            "bias": b_col,
        })
    return in_maps


def _unshard(parts):
    full = np.zeros((N, D), dtype=np.float32)
    for c in range(NCORES):
        perm = _PERM[c]
        valid = perm >= 0
        full[perm[valid]] = parts[c][valid]
    return full


def kernel(feat, src, dst, W, b):
    nc, idx_s, sel_s = _get_compiled(src, dst)
    in_maps = _make_in_maps(feat, src, dst, W, b, idx_s, sel_s)
    res = run_bass_kernel_spmd(nc, in_maps, list(range(NCORES)))
    parts = [res.results[c]["rstT"].T for c in range(NCORES)]
    return _unshard(parts)



# revision 2
# speedup vs baseline: 1.6711x; 1.6711x over previous
"""ChebConv (K=4) GNN message passing on 8 Trainium2 NeuronCores.

v2: bf16 message path + scaled-space recurrence.

Strategy (1D vertex partitioning, hardcoded for N=100000, E=1600000,
D_in=D_out=64, K=4, lambda_max=2.0):

  Node re-sharding: nodes are padded to NPAD=102400 and mapped
  n -> (g = n // 25600, j = n % 25600, c = j // 3200, r = j % 3200).
  Core c owns local row l = g*3200 + r (SHARD=12800 rows, 100 blocks of
  128). Groups g are the gather slabs: slab_g = AllGather over cores of
  their [3200, 128]-bf16 group-g rows (payload in cols 0:64, junk pad in
  64:128 so each gathered element is the SWDGE-required 256 bytes);
  slab row j = c*3200 + r < 25600 (int16-safe gather indices).

  Scaled-space recurrence: y_i = norm * Tx_i, so the PSUM drain directly
  produces the next AllGather payload:
    y_0 = feat * norm
    y_1 = -norm^2 * segsum(y_0)
    y_i = -2 norm^2 * segsum(y_{i-1}) - y_{i-2}
  and rst = (1/norm) * sum_k y_k @ W_k + b with y_0/norm = feat.

  Per propagation:
    - Edges are partitioned by dst owner and sorted by (src_group,
      dst_block); each 128-edge tile is fetched with chunked dma_gather
      (2048 idxs/call, round-robin over the 4 SWDGE queues; issue order
      is front-load-then-round-robin across groups so the in-order Pool
      engine never gates one group's prefetch on another group's
      consumption).
    - Selection matrices (sel[e,d] = local_dst[e]==d, bf16) are generated
      6 tiles per vector instruction via dual multi-dim broadcast
      is_equal, just-in-time in consumption order (per-group rings).
    - Segment-sum: per 8-dst-block span, one PSUM bank accumulates each
      block's tiles from all 4 groups in a single uninterrupted bf16
      matmul chain (interleaved open chains on a bank corrupt
      accumulation on real HW). The drain IS the recurrence: DVE reads
      the bank, scales by -norm^2 (and subtracts y_{i-2}), writing the
      bf16 y that feeds the next AllGather, which is pre-issued per
      node-group as soon as its last span drains.
  Final: rst^T = sum_k W_k^T @ Tx_k^T per block via bf16 PE transposes
  batched 4 blocks per PSUM bank, bf16 W matmuls, fused bias via
  tensor_scalar. feat stays resident in SBUF from the initial load.

The host permutes feat/deg in and un-permutes rstT out.
"""
import numpy as np
from contextlib import ExitStack

import concourse.bass as bass
import concourse.tile as tile
from concourse import bacc, mybir
from concourse.bass_utils import run_bass_kernel_spmd
from concourse.masks import make_identity

N = 100000
E = 1600000
D = 64
KORD = 4
NCORES = 8
P = 128
NGRP = 4
PER = 3200              # rows per core per group
GPB = PER // P          # 25 blocks per group
NBLK = NGRP * GPB       # 100 blocks per core
SHARD = NGRP * PER      # 12800 rows per core
GRP = NCORES * PER      # 25600 rows per slab (int16-safe)
NPAD = NGRP * GRP       # 102400
ROWW = 2 * D            # padded slab row width (bf16) = 256B gather elem
CHUNK_TILES = 8         # 1024 idxs per dma_gather call
MSG_BUFS = 6            # msg ring depth per source group
SELT = 6                # max tiles per batched sel-gen instruction
SPAN = 8                # dst blocks per PSUM bank span
PAD_SELCOL = 999.0


def _node_map(n):
    """global node id -> (core, local row)"""
    g = n // GRP
    j = n % GRP
    c = j // PER
    r = j % PER
    return c, g * PER + r


def _balanced_perm(src: np.ndarray, dst: np.ndarray):
    """Assign nodes to (core, local row) with degree-balanced blocks.

    Group membership g = n // GRP is kept (it fixes which slab a node's
    outgoing messages live in); within each group, nodes are packed into
    the NCORES*GPB blocks of 128 so that each block's in-degree from
    every source group stays near the 500-per-bucket mean (multi-dim LPT
    greedy). This collapses the per-(group, block) tile padding that a
    random layout needs (max-over-cores of a Poisson(500) spread).
    Returns perm[c, l] = global node id (or -1 for pad slots).
    """
    deg4 = np.zeros((N, NGRP), dtype=np.int64)
    np.add.at(deg4, (dst, src % NGRP), 1)
    nbins = NCORES * GPB
    perm = np.full((NCORES, SHARD), -1, dtype=np.int64)
    for g in range(NGRP):
        nodes = np.arange(g, N, NGRP)
        d = deg4[nodes].astype(np.float64)
        order = np.argsort(-d.sum(1), kind="stable")
        # pad the node list to nbins*P virtual slots (degree-0 fillers)
        npad_items = nbins * P - len(order)
        d_ext = np.concatenate([d, np.zeros((npad_items, NGRP))])
        order = np.concatenate([order, np.arange(len(order),
                                                 len(order) + npad_items)])
        loads = np.zeros((nbins, NGRP))
        assign = np.empty(nbins * P, dtype=np.int64)
        # rounds: each round places one item in every bin (fill stays
        # exact); within a round, items (desc by degree) greedily pick the
        # unused bin minimizing the projected max per-source-group load
        for r in range(P):
            items = order[r * nbins:(r + 1) * nbins]     # desc by degree
            score = np.empty(nbins)
            used_inf = np.zeros(nbins)
            for i in items:
                np.max(loads + d_ext[i], axis=1, out=score)
                score += used_inf
                b = int(np.argmin(score))
                assign[i] = b
                loads[b] += d_ext[i]
                used_inf[b] = np.inf
            used_inf[:] = 0.0
        # bin (c, b_local) -> rows l = g*PER + b_local*128 + slot on core c
        slot = np.zeros(nbins, dtype=np.int64)
        for i in np.argsort(assign[:len(nodes)], kind="stable"):
            b = assign[i]
            c, bl = divmod(b, GPB)
            perm[c, g * PER + bl * P + slot[b]] = nodes[i]
            slot[b] += 1
    return perm


def _prepare_edges(src: np.ndarray, dst: np.ndarray):
    """Partition + sort edges per core; build idx16/selcol streams."""
    perm = _balanced_perm(src, dst)
    # invert: node -> (core, local row)
    pos_c = np.full(NPAD, -1, dtype=np.int64)
    pos_l = np.full(NPAD, -1, dtype=np.int64)
    for c in range(NCORES):
        valid = perm[c] >= 0
        pos_c[perm[c][valid]] = c
        pos_l[perm[c][valid]] = np.nonzero(valid)[0]
    owner = pos_c[dst]
    ldst = pos_l[dst]                   # local row on owner core
    gsrc = src % NGRP                   # slab (group) of the source
    lsrc = pos_c[src] * PER + pos_l[src] % PER   # slab-local row
    b = ldst >> 7

    per_core = []
    counts = np.zeros((NCORES, NGRP, NBLK), dtype=np.int64)
    for c in range(NCORES):
        m = owner == c
        s, d, g, bb = lsrc[m], ldst[m], gsrc[m], b[m]
        order = np.lexsort((d, bb, g))
        s, d, g, bb = s[order], d[order], g[order], bb[order]
        np.add.at(counts[c], (g, bb), 1)
        per_core.append((s, d, g, bb))
    tiles_per = np.maximum(1, (counts.max(axis=0) + 127) // 128)  # [NGRP, NBLK]
    ntile = int(tiles_per.sum())
    nslot = ntile * P
    tile_start = np.zeros((NGRP, NBLK), dtype=np.int64)
    acc = 0
    for g in range(NGRP):
        for bb in range(NBLK):
            tile_start[g, bb] = acc
            acc += tiles_per[g, bb]

    idx_streams, sel_streams = [], []
    for c in range(NCORES):
        s, d, g, bb = per_core[c]
        flat_idx = np.zeros(nslot, dtype=np.int16)
        flat_sel = np.full(nslot, PAD_SELCOL, dtype=np.float32)
        cnt = counts[c]
        estart = np.concatenate([[0], np.cumsum(cnt.ravel())])
        for gg in range(NGRP):
            for bb2 in range(NBLK):
                k = gg * NBLK + bb2
                e0, e1 = estart[k], estart[k + 1]
                if e1 == e0:
                    continue
                s0 = tile_start[gg, bb2] * P
                n = e1 - e0
                flat_idx[s0:s0 + n] = s[e0:e1].astype(np.int16)
                flat_sel[s0:s0 + n] = (d[e0:e1] & 127).astype(np.float32)
        idx16 = np.tile(flat_idx.reshape(nslot // 16, 16).T, (8, 1))
        selcol = np.ascontiguousarray(flat_sel.reshape(ntile, P).T)
        idx_streams.append(np.ascontiguousarray(idx16))
        sel_streams.append(selcol)
    global _PERM
    _PERM = perm
    return tiles_per, tile_start, ntile, idx_streams, sel_streams


def _build_nc(tiles_per, tile_start, ntile, ablate=(),
              chunkt=CHUNK_TILES, msg_bufs=MSG_BUFS, selgen="tt",
              selt=SELT, sp=False, front=6):
    """ablate: dev-only subsets of {'nogather','noselmm','nocoll','q1',
    'nodrain'} for component timing; production call passes nothing."""
    ablate = frozenset(ablate)
    nc = bacc.Bacc("TRN2", target_bir_lowering=False, debug=False,
                   enable_asserts=True, num_devices=NCORES,
                   num_swdge_queues=4)
    f32 = mybir.dt.float32
    bf16 = mybir.dt.bfloat16
    nslot16 = ntile * P // 16

    feat_in = nc.dram_tensor("feat", [SHARD, D], f32, kind="ExternalInput").ap()
    deg_in = nc.dram_tensor("deg", [P, NBLK], f32, kind="ExternalInput").ap()
    idx_in = nc.dram_tensor("idx", [P, nslot16], mybir.dt.int16, kind="ExternalInput").ap()
    sel_in = nc.dram_tensor("selcol", [P, ntile], f32, kind="ExternalInput").ap()
    iota_in = nc.dram_tensor("iota", [P, P], bf16, kind="ExternalInput").ap()
    w_in = nc.dram_tensor("w", [D, KORD * D], bf16, kind="ExternalInput").ap()
    b_in = nc.dram_tensor("bias", [D, 1], f32, kind="ExternalInput").ap()
    out = nc.dram_tensor("rstT", [D, SHARD], f32, kind="ExternalOutput").ap()

    # span layout over the 100 blocks: 12 spans of 8 + 1 span of 4
    spans = []
    b0 = 0
    while b0 < NBLK:
        spans.append((b0, min(b0 + SPAN, NBLK)))
        b0 += SPAN
    # last span fully covering each group's final block
    grp_last_span = [max(si for si, (s0, s1) in enumerate(spans)
                         if s0 < (g + 1) * GPB) for g in range(NGRP)]

    with tile.TileContext(nc) as tc:
        with ExitStack() as ctx:
            const = ctx.enter_context(tc.tile_pool(name="const", bufs=1))
            typ = ctx.enter_context(tc.tile_pool(name="typ", bufs=1))
            msgp = ctx.enter_context(tc.tile_pool(name="msgp", bufs=msg_bufs))
            selp = ctx.enter_context(tc.tile_pool(name="selp", bufs=2))
            psum = ctx.enter_context(tc.tile_pool(name="psum", bufs=4, space="PSUM"))
            trp = ctx.enter_context(tc.tile_pool(name="trp", bufs=2, space="PSUM"))
            rstp = ctx.enter_context(tc.tile_pool(name="rstp", bufs=2, space="PSUM"))
            outp = ctx.enter_context(tc.tile_pool(name="outp", bufs=1))
            txup = ctx.enter_context(tc.tile_pool(name="txup", bufs=2))
            dram = ctx.enter_context(tc.tile_pool(name="dram", bufs=1, space="DRAM"))

            # ---- constants / streams
            idx_sb = const.tile([P, nslot16], mybir.dt.int16)
            nc.sync.dma_start(idx_sb[:], idx_in[:])
            sel_sb = const.tile([P, ntile], f32)
            nc.sync.dma_start(sel_sb[:], sel_in[:])
            iota_sb = const.tile([P, P], bf16)
            nc.sync.dma_start(iota_sb[:], iota_in[:])
            w_sb = const.tile([D, KORD * D], bf16)
            nc.sync.dma_start(w_sb[:], w_in[:])
            b_sb = const.tile([D, 1], f32)
            nc.sync.dma_start(b_sb[:], b_in[:])
            ident = const.tile([P, P], bf16)
            make_identity(nc, ident[:])

            # ---- norm family from deg: norm = clip(deg,1)^-1/2,
            #      m2 = -norm^2, m4 = -2 norm^2, recipn = clip(deg,1)^+1/2
            deg_sb = const.tile([P, NBLK], f32)
            nc.sync.dma_start(deg_sb[:], deg_in[:])
            degc = const.tile([P, NBLK], f32)
            nc.vector.tensor_scalar_max(degc[:], deg_sb[:], 1.0)
            recipn = const.tile([P, NBLK], f32)
            nc.scalar.activation(recipn[:], degc[:], mybir.ActivationFunctionType.Sqrt)
            norm = const.tile([P, NBLK], f32)
            nc.vector.reciprocal(norm[:], recipn[:])
            m2 = const.tile([P, NBLK], f32)
            nc.vector.reciprocal(m2[:], degc[:])
            nc.vector.tensor_scalar_mul(m2[:], m2[:], -1.0)
            m4 = const.tile([P, NBLK], f32)
            nc.vector.tensor_scalar_mul(m4[:], m2[:], 2.0)

            # ---- feat resident + y ring (node-major [p, b*64+f], bf16)
            feat_sb = const.tile([P, NBLK * D], f32)
            nc.sync.dma_start(
                feat_sb[:].rearrange("p (b f) -> p b f", b=NBLK),
                feat_in.rearrange("(b p) f -> p b f", p=P))
            yA = typ.tile([P, NBLK * D], bf16)   # y_0 -> later y_3
            yB = typ.tile([P, NBLK * D], bf16)   # y_1
            yC = typ.tile([P, NBLK * D], bf16)   # y_2
            y_ring = [yA, yB, yC]

            table_own = [[dram.tile([PER, ROWW], bf16, name=f"town{pp}_{g}")
                          for g in range(NGRP)] for pp in range(KORD - 1)]
            slabs = [[dram.tile([GRP, ROWW], bf16, addr_space="Shared",
                                name=f"slab{pp}_{g}")
                      for g in range(NGRP)] for pp in range(KORD - 1)]

            def emit_ag(pp, g, y_src):
                """y_src[group g] (bf16) -> padded DRAM table -> AllGather."""
                c0, c1 = g * GPB * D, (g + 1) * GPB * D
                nc.sync.dma_start(
                    table_own[pp - 1][g].opt()
                        .rearrange("(b p) f -> p b f", p=P)[:, :, 0:D],
                    y_src[:, c0:c1].rearrange("p (b f) -> p b f", b=GPB))
                if "nocoll" in ablate:
                    nc.sync.dma_start(
                        slabs[pp - 1][g].opt()[0:PER, :],
                        table_own[pp - 1][g].opt()[:, :])
                else:
                    nc.gpsimd.collective_compute(
                        "AllGather", mybir.AluOpType.bypass,
                        replica_groups=[list(range(NCORES))],
                        ins=[table_own[pp - 1][g].opt()],
                        outs=[slabs[pp - 1][g].opt()],
                    )

            # y_0 = feat * norm (per group, bf16 out), AG per group asap
            for g in range(NGRP):
                c0, c1 = g * GPB * D, (g + 1) * GPB * D
                nc.vector.tensor_tensor(
                    out=yA[:, c0:c1].rearrange("p (b f) -> p b f", b=GPB),
                    in0=feat_sb[:, c0:c1].rearrange("p (b f) -> p b f", b=GPB),
                    in1=norm[:, g * GPB:(g + 1) * GPB]
                        .rearrange("p (b o) -> p b o", o=1)
                        .to_broadcast([P, GPB, D]),
                    op=mybir.AluOpType.mult,
                )
                emit_ag(1, g, yA)

            for prop in range(1, KORD):
                y_new = y_ring[prop % 3]
                y_prev2 = y_ring[(prop - 2) % 3]
                if "noselmm" in ablate or "nodrain" in ablate:
                    nc.vector.memset(y_new[:], 0.0)

                # gathers for all groups (waits AG_g via slab deps); issue
                # order is front-load-then-round-robin so the in-order Pool
                # engine never gates one group's prefetch on another group's
                # consumption (deadlocks otherwise)
                chunk_lists = []
                for g in range(NGRP):
                    gt0 = int(tile_start[g, 0])
                    gt1 = int(tile_start[g, NBLK - 1] + tiles_per[g, NBLK - 1])
                    cl = []
                    j = gt0
                    while j < gt1:
                        cnt = min(chunkt, gt1 - j)
                        cl.append((j, cnt))
                        j += cnt
                    chunk_lists.append(cl)
                issue = []
                FRONT = front
                for g in range(NGRP):
                    for c in range(min(FRONT, len(chunk_lists[g]))):
                        issue.append((g, c))
                rrmax = max(len(cl) for cl in chunk_lists)
                for c in range(FRONT, rrmax):
                    for g in range(NGRP):
                        if c < len(chunk_lists[g]):
                            issue.append((g, c))
                chunk_tiles = {}
                for qi, (g, ci) in enumerate(issue):
                    j, cnt = chunk_lists[g][ci]
                    tbl = slabs[prop - 1][g].opt()
                    m = msgp.tile([P, chunkt, ROWW], bf16, tag=f"msg{g}",
                                  name=f"m{prop}_{g}_{ci}")
                    if "nogather" in ablate:
                        nc.vector.memset(m[:, 0:cnt, :], 0.25)
                    else:
                        nc.gpsimd.dma_gather(
                            out_ap=m[:, 0:cnt, :],
                            in_ap=tbl[:, :],
                            idxs_ap=idx_sb[:, j * 8:(j + cnt) * 8],
                            num_idxs=cnt * P,
                            num_idxs_reg=cnt * P,
                            elem_size=ROWW,
                            single_packet=sp or ("sp1" in ablate),
                            queue_num=0 if "q1" in ablate else qi % 4,
                        )
                    for jl in range(cnt):
                        chunk_tiles[j + jl] = (m, jl)

                if "noselmm" in ablate:
                    if prop < KORD - 1:
                        for g in range(NGRP):
                            emit_ag(prop + 1, g, y_new)
                    continue
                # span-major, block-contiguous chains: for each block the 4
                # groups' tiles accumulate in ONE uninterrupted PSUM chain
                # (interleaved open chains on a bank corrupt accumulation on
                # HW); sel batches are generated just-in-time in consumption
                # order; the drain IS the recurrence (DVE reads the bank,
                # writes bf16 y directly)
                for si, (s0, s1) in enumerate(spans):
                    bank = psum.tile([P, SPAN * D], f32, tag="ps",
                                     name=f"ps{prop}_{si}")
                    sel_tiles = {}
                    gen_pos = []
                    for g in range(NGRP):
                        gen_pos.append(int(tile_start[g, s0]))
                    span_end = [int(tile_start[g, s1 - 1] + tiles_per[g, s1 - 1])
                                for g in range(NGRP)]
                    for b in range(s0, s1):
                        for g in range(NGRP):
                            tb = int(tiles_per[g, b])
                            j0 = int(tile_start[g, b])
                            while gen_pos[g] < j0 + tb:
                                jg = gen_pos[g]
                                cnt = min(selt, span_end[g] - jg)
                                sel = selp.tile([P, selt, P], bf16, tag=f"sel{g}",
                                                name=f"sel{prop}_{si}_{g}_{jg}")
                                if selgen == "ts":
                                    for jl in range(cnt):
                                        nc.vector.tensor_scalar(
                                            out=sel[:, jl, :], in0=iota_sb[:],
                                            scalar1=sel_sb[:, jg + jl:jg + jl + 1],
                                            scalar2=None,
                                            op0=mybir.AluOpType.is_equal,
                                        )
                                else:
                                    nc.vector.tensor_tensor(
                                        out=sel[:, 0:cnt, :],
                                        in0=sel_sb[:, jg:jg + cnt]
                                            .rearrange("p (t o) -> p t o", o=1)
                                            .to_broadcast([P, cnt, P]),
                                        in1=iota_sb[:]
                                            .rearrange("p (o d) -> p o d", o=1)
                                            .to_broadcast([P, cnt, P]),
                                        op=mybir.AluOpType.is_equal,
                                    )
                                for jl in range(cnt):
                                    sel_tiles[jg + jl] = (sel, jl)
                                gen_pos[g] = jg + cnt
                            sl = bank[:, (b - s0) * D:(b - s0 + 1) * D]
                            for t in range(tb):
                                jj = j0 + t
                                m, jl = chunk_tiles[jj]
                                sel, sjl = sel_tiles[jj]
                                nc.tensor.matmul(
                                    sl, lhsT=sel[:, sjl, :], rhs=m[:, jl, 0:D],
                                    start=(g == 0 and t == 0),
                                    stop=(g == NGRP - 1 and t == tb - 1),
                                    skip_group_check=True,
                                )
                    if "nodrain" in ablate:
                        if prop < KORD - 1 and si == len(spans) - 1:
                            for g in range(NGRP):
                                emit_ag(prop + 1, g, y_new)
                        continue
                    nb = s1 - s0
                    scale = m2 if prop == 1 else m4
                    nc.vector.tensor_tensor(
                        out=y_new[:, s0 * D:s1 * D]
                            .rearrange("p (b f) -> p b f", b=nb),
                        in0=bank[:, 0:nb * D]
                            .rearrange("p (b f) -> p b f", b=nb),
                        in1=scale[:, s0:s1]
                            .rearrange("p (b o) -> p b o", o=1)
                            .to_broadcast([P, nb, D]),
                        op=mybir.AluOpType.mult,
                    )
                    if prop >= 2:
                        nc.vector.tensor_tensor(
                            out=y_new[:, s0 * D:s1 * D],
                            in0=y_new[:, s0 * D:s1 * D],
                            in1=y_prev2[:, s0 * D:s1 * D],
                            op=mybir.AluOpType.subtract,
                        )
                    # pre-issue next prop's AllGather for any node-group whose
                    # blocks are now final, hiding the collective behind the
                    # remaining spans
                    if prop < KORD - 1:
                        for gq in range(NGRP):
                            if grp_last_span[gq] == si:
                                emit_ag(prop + 1, gq, y_new)

            # ---- rst^T = (sum_k W_k^T @ Tx_k^T) with Tx_k = recipn*y_k
            #      (k=0: feat), 4 blocks per PSUM bank, all bf16
            FB = 4  # blocks per output bank
            for q in range(NBLK // FB):
                blocks = range(q * FB, (q + 1) * FB)
                c0, c1 = q * FB * D, (q + 1) * FB * D
                txu = txup.tile([P, KORD, FB, D], bf16, tag="txu", name=f"txu{q}")
                # k=0: feat (unscaled), bf16 convert on scalar engine
                nc.scalar.activation(
                    txu[:, 0, :, :],
                    feat_sb[:, c0:c1].rearrange("p (b f) -> p b f", b=FB),
                    mybir.ActivationFunctionType.Copy)
                for k in range(1, KORD):
                    nc.vector.tensor_tensor(
                        out=txu[:, k, :, :],
                        in0=y_ring[k % 3][:, c0:c1]
                            .rearrange("p (b f) -> p b f", b=FB),
                        in1=recipn[:, q * FB:(q + 1) * FB]
                            .rearrange("p (b o) -> p b o", o=1)
                            .to_broadcast([P, FB, D]),
                        op=mybir.AluOpType.mult,
                    )
                rst_ps = rstp.tile([D, FB * P], f32, tag="rst", name=f"rst{q}")
                for bi, b in enumerate(blocks):
                    tr_ps = trp.tile([D, KORD * P], bf16, tag="tr", name=f"tr{b}")
                    for k in range(KORD):
                        nc.tensor.transpose(
                            tr_ps[:, k * P:(k + 1) * P], txu[:, k, bi, :],
                            ident[:])
                    txT = outp.tile([D, KORD * P], bf16, tag="txT", name=f"txT{b}")
                    nc.scalar.activation(
                        txT[:], tr_ps[:], mybir.ActivationFunctionType.Copy)
                    for k in range(KORD):
                        nc.tensor.matmul(
                            rst_ps[:, bi * P:(bi + 1) * P],
                            lhsT=w_sb[:, k * D:(k + 1) * D],
                            rhs=txT[:, k * P:(k + 1) * P],
                            start=(k == 0), stop=(k == KORD - 1),
                            skip_group_check=True,
                        )
                ostage = outp.tile([D, FB * P], f32, tag="ostage", name=f"os{q}")
                nc.vector.tensor_scalar(
                    out=ostage[:], in0=rst_ps[:], scalar1=b_sb[:, 0:1],
                    scalar2=None, op0=mybir.AluOpType.add)
                nc.sync.dma_start(out[:, q * FB * P:(q + 1) * FB * P], ostage[:])
    nc.compile()
    return nc


_CACHE = {}


def _get_compiled(src: np.ndarray, dst: np.ndarray):
    key = (src.tobytes()[:256], dst.tobytes()[:256], len(src))
    if key not in _CACHE:
        tpb, ts, ntile, idx_s, sel_s = _prepare_edges(src, dst)
        nc = _build_nc(tpb, ts, ntile)
        _CACHE[key] = (nc, idx_s, sel_s)
    return _CACHE[key]


def _perm_rows():
    """perm[c, l] = global node id for core c local row l (or -1 if pad)."""
    l = np.arange(SHARD)
    g = l // PER
    r = l % PER
    perm = np.empty((NCORES, SHARD), dtype=np.int64)
    for c in range(NCORES):
        n = g * GRP + c * PER + r
        perm[c] = np.where(n < N, n, -1)
    return perm


_PERM = _perm_rows()


def _make_in_maps(feat, src, dst, W, b, idx_s, sel_s):
    deg_full = np.bincount(dst, minlength=N).astype(np.float32)
    iota = np.broadcast_to(np.arange(P, dtype=np.float32)[None, :], (P, P))
    w_flat = np.ascontiguousarray(
        W.astype(np.float32).transpose(1, 0, 2).reshape(D, KORD * D))
    b_col = np.ascontiguousarray(b.astype(np.float32).reshape(D, 1))
    try:
        import ml_dtypes
        bf = ml_dtypes.bfloat16
    except ImportError:
        bf = np.float32
    in_maps = []
    for c in range(NCORES):
        perm = _PERM[c]
        valid = perm >= 0
        feat_c = np.zeros((SHARD, D), dtype=np.float32)
        feat_c[valid] = feat[perm[valid]]
        deg_c = np.zeros(SHARD, dtype=np.float32)
        deg_c[valid] = deg_full[perm[valid]]
        in_maps.append({
            "feat": feat_c,
            "deg": np.ascontiguousarray(deg_c.reshape(NBLK, P).T),
            "idx": idx_s[c],
            "selcol": sel_s[c],
            "iota": iota.astype(bf),
            "w": w_flat.astype(bf),
            "bias": b_col,
        })
    return in_maps


def _unshard(parts):
    full = np.zeros((N, D), dtype=np.float32)
    for c in range(NCORES):
        perm = _PERM[c]
        valid = perm >= 0
        full[perm[valid]] = parts[c][valid]
    return full


def kernel(feat, src, dst, W, b):
    nc, idx_s, sel_s = _get_compiled(src, dst)
    in_maps = _make_in_maps(feat, src, dst, W, b, idx_s, sel_s)
    res = run_bass_kernel_spmd(nc, in_maps, list(range(NCORES)))
    parts = [res.results[c]["rstT"].T for c in range(NCORES)]
    return _unshard(parts)
